# revision 1
# baseline (speedup 1.0000x reference)
"""Trainium2 Bass kernel for tied-row axial attention (MSA row attention).

Reference computation (B=1, M=128 rows, N=256 residues, D=256, H=8, DH=64):
    xn   = LayerNorm_D(x) * ln_g + ln_b
    bias = einsum('bijc,ch->bhij', edges, Wb)
    q    = (xn @ Wq).heads * DH**-0.5 ; k, v = (xn @ Wkv).heads
    qm   = q.mean(axis=m)                       (tied queries)
    dots = einsum('bihd,bmjhd->bmhij', qm, k) + bias
    attn = softmax_j(dots)                      (mask is all-ones)
    out  = (attn @ v  * sigmoid(xn @ Wg + bg)) @ Wo + bo

Distribution (8 cores): shard MSA rows m (16/core).  The tied-query mean
needs one AllReduce of sum_m(xn^T) [256,256].  The pair bias is sharded
over the i axis (32/core) and AllGathered (2 MB).

Per-core dataflow (all matmuls fp32r):
  - LN in natural [n,d] layout (bn_stats) -> PE-transpose -> xn^T [d,n].
  - k^T,g^T in [e,n] layout (lhsT=W), v in [n,e] layout (lhsT=xn^T).
  - S^T[j,i] per (m,h): row-packed K=64 head-pair matmuls
    (lhsT = k^T slice, rhs = qm^T slice, base partitions 0/64).
  - softmax: ACT exp(S^T) -> GPSIMD multiply by exp(bias^T) -> AV matmul
    with a ones column appended to v, so row 64 of the AV output is the
    softmax denominator.  Normalization uses DMA gather of the sums rows,
    DVE reciprocal, and a DMA partition-broadcast via a DRAM bounce.
  - gated output: t = (O/sums + bv) * sigmoid(g^T+bg); y^T = Wo^T @ t + bo.
  - y is written transposed [m, d, n]; the host un-transposes.
"""

import numpy as np

import concourse.bass as bass
import concourse.tile as tile
import concourse.mybir as mybir
from concourse.masks import make_identity

F32 = mybir.dt.float32
F32R = mybir.dt.float32r
BF16 = mybir.dt.bfloat16
AF = mybir.ActivationFunctionType
ALU = mybir.AluOpType

# problem dims (hardcoded per contract)
B, M, N, D = 1, 128, 256, 256
DE = 128
H, DH = 8, 64
INNER = H * DH          # 512
NCORES = 8
M_LOC = M // NCORES     # 16 rows per core
I_LOC = N // NCORES     # 32 bias-i per core
NPAIR = H // 2          # 4 head pairs
EPS = 1e-5


def _split_multi_waits(nc, cap: int = 1):
    """This container's walrus accepts at most one sync-wait per instruction;
    spill extra Tile-emitted waits onto standalone NOPs on the same engine
    (same-engine sequential waits are semantically identical to a wait list)."""
    for f in nc.m.functions:
        for bb in f.blocks:
            out = []
            for ins in bb.instructions:
                si = ins.sync_info
                waits = list(si.on_wait) if (si is not None and si.on_wait) else []
                if len(waits) > cap:
                    spill, keep = waits[:-cap], waits[-cap:]
                    k = 0
                    while spill:
                        chunk, spill = spill[:cap], spill[cap:]
                        nop = mybir.InstNoOp(name=f"{ins.name}-sw{k}", ins=[], outs=[])
                        nop.engine = ins.engine
                        nop.sync_info = mybir.SyncInfo(on_wait=chunk, on_update=[])
                        out.append(nop)
                        k += 1
                    si.on_wait = keep
                out.append(ins)
            bb.instructions = out


def build_program(n_cores: int = NCORES, m_loc: int = M_LOC, proj_pipe: int = 4,
                  split_waits: bool = True):
    """Build the SPMD Bass program (identical on every core)."""
    i_loc = N // n_cores
    n_edge_tiles = (i_loc * N) // 128     # [128,128] edge tiles per core

    nc = bass.Bass()

    x_in = nc.dram_tensor("x", [m_loc, N, D], F32, kind="ExternalInput")
    e_in = nc.dram_tensor("edges", [i_loc * N, DE], F32, kind="ExternalInput")
    wk_in = nc.dram_tensor("wk", [D, INNER], F32, kind="ExternalInput")
    wv_in = nc.dram_tensor("wv", [D, INNER], F32, kind="ExternalInput")
    wg_in = nc.dram_tensor("wg", [D, INNER], F32, kind="ExternalInput")
    wq_in = nc.dram_tensor("wq", [D, INNER], F32, kind="ExternalInput")
    wo_in = nc.dram_tensor("wo", [INNER, D], F32, kind="ExternalInput")
    wb_in = nc.dram_tensor("wb", [DE, H], F32, kind="ExternalInput")
    sel_in = nc.dram_tensor("sel", [NPAIR, H, 128], F32, kind="ExternalInput")
    bk_in = nc.dram_tensor("bk", [INNER], F32, kind="ExternalInput")
    bv_in = nc.dram_tensor("bv", [INNER], F32, kind="ExternalInput")
    bg_in = nc.dram_tensor("bg", [INNER], F32, kind="ExternalInput")
    bq_in = nc.dram_tensor("bq", [INNER], F32, kind="ExternalInput")
    bo_in = nc.dram_tensor("bo", [D], F32, kind="ExternalInput")
    y_out = nc.dram_tensor("y", [m_loc, D, N], F32, kind="ExternalOutput")

    groups = [list(range(n_cores))]

    def r(ap):
        return ap

    with tile.TileContext(nc) as tc:
        with tc.tile_pool(name="consts", bufs=1) as consts, \
             tc.tile_pool(name="persist", bufs=1) as persist, \
             tc.tile_pool(name="psum", bufs=1, space="PSUM") as psum, \
             tc.tile_pool(name="dram", bufs=1, space="DRAM") as dram:

            # ---------------- constants / weights to SBUF ----------------
            ident = consts.tile([128, 128], F32)
            make_identity(nc, ident)
            eps_sb = consts.tile([128, 1], F32)
            nc.vector.memset(eps_sb, EPS)
            ones_f = consts.tile([128, 1], F32)
            nc.vector.memset(ones_f, 1.0)
            ones_r = consts.tile([128, 1], F32R)
            nc.vector.tensor_copy(out=ones_r, in_=ones_f)

            def load_w_dke(dram_t, name):
                # [D, INNER] -> sbuf [128, 2(dblk), INNER] (cast to f32r)
                t = consts.tile([128, D // 128, INNER], F32R, name=name)
                dap = dram_t[:]
                src = bass.AP(tensor=dap.tensor, offset=dap.offset,
                              ap=[[INNER, 128], [INNER * 128, D // 128], [1, INNER]])
                nc.gpsimd.dma_start(out=t, in_=src)
                return t

            wk_sb = load_w_dke(wk_in, "wk_sb")
            wv_sb = load_w_dke(wv_in, "wv_sb")
            wg_sb = load_w_dke(wg_in, "wg_sb")
            wq_sb = load_w_dke(wq_in, "wq_sb")

            wo_sb = consts.tile([128, INNER // 128, D], F32R)
            wo_ap = wo_in[:]
            nc.gpsimd.dma_start(
                out=wo_sb,
                in_=bass.AP(tensor=wo_ap.tensor, offset=wo_ap.offset,
                            ap=[[D, 128], [D * 128, INNER // 128], [1, D]]))
            wb_sb = consts.tile([DE, H], F32R)
            nc.gpsimd.dma_start(out=wb_sb, in_=wb_in[:])
            sel_sb = consts.tile([H, NPAIR, 128], F32R)
            sel_ap = sel_in[:]
            nc.gpsimd.dma_start(
                out=sel_sb,
                in_=bass.AP(tensor=sel_ap.tensor, offset=sel_ap.offset,
                            ap=[[128, H], [H * 128, NPAIR], [1, 128]]))

            def load_bias(dram_t, nblk, name):
                t = consts.tile([128, nblk], F32, name=name)
                dap = dram_t[:]
                src = bass.AP(tensor=dap.tensor, offset=dap.offset,
                              ap=[[1, 128], [128, nblk]])
                nc.sync.dma_start(out=t, in_=src)
                return t

            bk_sb = load_bias(bk_in, 4, "bk_sb")
            bv_sb = load_bias(bv_in, 4, "bv_sb")
            bg_sb = load_bias(bg_in, 4, "bg_sb")
            bq_sb = load_bias(bq_in, 4, "bq_sb")
            bo_sb = load_bias(bo_in, 2, "bo_sb")

            # ---------------- persistent activations ----------------
            # xn^T for all local rows: [128, m, dblk, n]
            xnT = persist.tile([128, m_loc, 2, N], F32R)
            vsum = persist.tile([128, 2, N], F32)     # sum_m xn^T (local)
            nc.vector.memset(vsum, 0.0)
            xnmT = persist.tile([128, 2, N], F32R)    # allreduced sum
            qmT = persist.tile([128, NPAIR, N], F32R)  # tied queries ^T
            ebt = persist.tile([128, 2, H, N], F32)   # exp(bias^T)
            bias_loc = persist.tile([128, 2 * H * i_loc], BF16)

            # ---------------- phase 1a: pair bias (i-sharded) ----------------
            with tc.tile_pool(name="p1", bufs=6) as p1, \
                 tc.tile_pool(name="p1s", bufs=8) as p1s, \
                 tc.tile_pool(name="xp", bufs=1) as xp:
                x_all = xp.tile([128, m_loc, 2, D], F32)

                for m in range(m_loc):
                    for nb in range(2):
                        nc.sync.dma_start(
                            out=x_all[:, m, nb, :],
                            in_=x_in[m, nb * 128:(nb + 1) * 128, :])

                def edge_tile(t_i):
                    et = p1.tile([128, DE], F32, name="et")
                    nc.sync.dma_start(out=et,
                                      in_=e_in[t_i * 128:(t_i + 1) * 128, :])
                    etp = psum.tile([128, 128], F32, tag="mm", bufs=2, name="etp")
                    nc.tensor.transpose(etp, et, ident)
                    edT = p1.tile([128, 128], F32R, name="edT")
                    nc.vector.tensor_copy(out=edT, in_=etp)
                    bps = psum.tile([128, H], F32, tag="sp", bufs=2, name="bps")
                    nc.tensor.matmul(out=bps, lhsT=r(edT), rhs=r(wb_sb),
                                     start=True, stop=True)
                    # tile t_i = (i_l = t_i//2, jhalf = t_i%2)
                    i_l, jh = t_i // 2, t_i % 2
                    dst = bias_loc.rearrange("p (a h i) -> p a h i", a=2, h=H)
                    nc.vector.tensor_copy(out=dst[:, jh, :, i_l], in_=bps)

                def ln_tile(m, nb):
                    xv = x_all[:, m, nb, :]
                    stats = p1s.tile([128, 6], F32)
                    nc.vector.bn_stats(out=stats, in_=xv)
                    mv = p1s.tile([128, 2], F32)
                    nc.vector.bn_aggr(out=mv, in_=stats)
                    rstd = p1s.tile([128, 1], F32)
                    nc.scalar.activation(out=rstd, in_=mv[:, 1:2], func=AF.Sqrt,
                                         bias=eps_sb)
                    nc.vector.reciprocal(out=rstd, in_=rstd)
                    nmu = p1s.tile([128, 1], F32)
                    nc.vector.scalar_tensor_tensor(
                        out=nmu, in0=mv[:, 0:1], scalar=-1.0, in1=rstd,
                        op0=ALU.mult, op1=ALU.mult)
                    xnat = p1.tile([128, D], F32)
                    nc.scalar.activation(out=xnat, in_=xv, func=AF.Identity,
                                         bias=nmu, scale=rstd)
                    for db in range(2):
                        tps = psum.tile([128, 128], F32, tag="mx", bufs=2,
                                        name="tps")
                        nc.tensor.transpose(tps, xnat[:, db * 128:(db + 1) * 128],
                                            ident)
                        nc.vector.tensor_copy(
                            out=xnT[:, m, db, nb * 128:(nb + 1) * 128], in_=tps)
                        sl = vsum[:, db, nb * 128:(nb + 1) * 128]
                        nc.vector.tensor_add(out=sl, in0=sl, in1=tps)

                # LN-major interleave: LN gates the AllReduce, so it goes
                # first; one edge tile rides along with each LN tile.
                ln_jobs = [(m, nb) for m in range(m_loc) for nb in range(2)]
                for k in range(len(ln_jobs)):
                    ln_tile(*ln_jobs[k])
                    if k < n_edge_tiles:
                        edge_tile(k)
                for t_i in range(len(ln_jobs), n_edge_tiles):
                    edge_tile(t_i)

            # ---------------- phase 2: collectives ----------------
            # AllReduce first: it gates qm -> S -> the whole row pipeline.
            xs_d = dram.tile([2 * 128 * N], BF16)
            nc.gpsimd.dma_start(out=xs_d, in_=vsum)
            xr_d = dram.tile([2 * 128 * N], BF16, addr_space="Shared")
            nc.gpsimd.collective_compute(
                "AllReduce", ALU.add, replica_groups=groups,
                ins=[xs_d[:]], outs=[xr_d[:]])
            nc.gpsimd.dma_start(
                out=xnmT,
                in_=bass.AP(tensor=xr_d.tensor, offset=xr_d.offset,
                            ap=[[2 * N, 128], [N, 2], [1, N]]))

            bl_d = dram.tile([128 * 2 * H * i_loc], BF16)
            nc.gpsimd.dma_start(out=bl_d, in_=bias_loc)
            bg_d = dram.tile([n_cores * 128 * 2 * H * i_loc], BF16,
                             addr_space="Shared")
            nc.gpsimd.collective_compute(
                "AllGather", ALU.bypass, replica_groups=groups,
                ins=[bl_d[:]], outs=[bg_d[:]])

            # bias^T gather readback (bf16) staging AP (issued later)
            core_stride = 128 * 2 * H * i_loc
            src = bass.AP(tensor=bg_d.tensor, offset=bg_d.offset,
                          ap=[[2 * H * i_loc, 128], [H * i_loc, 2], [i_loc, H],
                              [core_stride, n_cores], [1, i_loc]])
            ebt_bf = persist.tile([128, 2, H, N], BF16)

            # ---------------- phase 4: pipelined projections + attention ----
            with tc.tile_pool(name="kT", bufs=proj_pipe) as kT_pool, \
                 tc.tile_pool(name="gT", bufs=proj_pipe) as gT_pool, \
                 tc.tile_pool(name="vo", bufs=proj_pipe) as vo_pool, \
                 tc.tile_pool(name="att", bufs=3) as att, \
                 tc.tile_pool(name="att2", bufs=6) as att2, \
                 tc.tile_pool(name="smal", bufs=3) as smal, \
                 tc.tile_pool(name="rdram", bufs=4, space="DRAM") as rdram:

                def proj(m):
                    kT = kT_pool.tile([128, NPAIR, N], F32R, name="kT")
                    gT = gT_pool.tile([128, NPAIR, N], F32, name="gT")
                    vo = vo_pool.tile([128, 2, H, DH + 1], F32R, name="vo")
                    for half in range(2):          # e-blocks (2 per psum bank)
                        kps = psum.tile([128, 2 * N], F32, tag="mm", bufs=2,
                                        name="kps")
                        gps = psum.tile([128, 2 * N], F32, tag="mm", bufs=2,
                                        name="gps")
                        for sub in range(2):
                            eb = half * 2 + sub
                            for db in range(2):
                                nc.tensor.matmul(
                                    out=kps[:, sub * N:(sub + 1) * N],
                                    lhsT=r(wk_sb[:, db, eb * 128:(eb + 1) * 128]),
                                    rhs=r(xnT[:, m, db, :]),
                                    start=(db == 0), stop=(db == 1))
                            for db in range(2):
                                nc.tensor.matmul(
                                    out=gps[:, sub * N:(sub + 1) * N],
                                    lhsT=r(wg_sb[:, db, eb * 128:(eb + 1) * 128]),
                                    rhs=r(xnT[:, m, db, :]),
                                    start=(db == 0), stop=(db == 1))
                        for sub in range(2):
                            eb = half * 2 + sub
                            nc.vector.tensor_scalar_add(
                                out=kT[:, eb, :], in0=kps[:, sub * N:(sub + 1) * N],
                                scalar1=bk_sb[:, eb:eb + 1])
                            gtmp = att.tile([128, N], F32, tag="gtmp")
                            nc.scalar.activation(
                                out=gtmp, in_=gps[:, sub * N:(sub + 1) * N],
                                func=AF.Tanh, bias=bg_sb[:, eb:eb + 1], scale=0.5)
                            nc.gpsimd.tensor_scalar(
                                out=gT[:, eb, :], in0=gtmp, scalar1=0.5,
                                scalar2=0.5, op0=ALU.mult, op1=ALU.add)
                    for nb in range(2):            # v natural [n, e]
                        vps = psum.tile([128, INNER], F32, tag="mm", bufs=2,
                                        name="vps")
                        for db in range(2):
                            nc.tensor.matmul(
                                out=vps,
                                lhsT=r(xnT[:, m, db, nb * 128:(nb + 1) * 128]),
                                rhs=r(wv_sb[:, db, :]),
                                start=(db == 0), stop=(db == 1))
                        nc.vector.tensor_copy(
                            out=vo[:, nb, :, 0:DH],
                            in_=vps.rearrange("p (h d) -> p h d", h=H))
                        ones_bc = bass.AP(tensor=ones_r.tensor, offset=ones_r.offset,
                                          ap=[ones_r.ap[0], [0, H], [1, 1]])
                        nc.vector.tensor_copy(out=vo[:, nb, :, DH:DH + 1],
                                              in_=ones_bc)
                    return kT, gT, vo

                def attn(m, kT, gT, vo):
                    usb_row = att2.tile([DH + 1, H, N], F32, tag="u", bufs=2,
                                        name="usb_row")
                    for pr in range(NPAIR):
                        he, ho = 2 * pr, 2 * pr + 1
                        sps_e = psum.tile([128, 2 * N], F32, tag="sp", bufs=2,
                                          name="sps_e")
                        sps_o = psum.tile([128, 2 * N], F32, tag="sp", bufs=2,
                                          name="sps_o")
                        for jb in range(2):
                            nc.tensor.matmul(
                                out=sps_e[:, jb * N:(jb + 1) * N],
                                lhsT=r(kT[0:64, pr, jb * 128:(jb + 1) * 128]),
                                rhs=r(qmT[0:64, pr, :]),
                                start=True, stop=True)
                            nc.tensor.matmul(
                                out=sps_o[:, jb * N:(jb + 1) * N],
                                lhsT=r(kT[64:128, pr, jb * 128:(jb + 1) * 128]),
                                rhs=r(qmT[64:128, pr, :]),
                                start=True, stop=True)
                        avps = psum.tile([DH + 1, 2, N], F32, tag="mx",
                                         bufs=2, name="avps")
                        for hi, h in enumerate((he, ho)):
                            ex = att.tile([128, 2, N], F32, tag="ex")
                            nc.scalar.activation(out=ex, in_=(
                                sps_e if hi == 0 else sps_o).rearrange(
                                "p (a n) -> p a n", a=2), func=AF.Exp)
                            aw = att.tile([128, 2, N], F32R, tag="aw")
                            nc.gpsimd.tensor_mul(out=aw, in0=ex,
                                                 in1=ebt[:, :, h, :])
                            for jb in range(2):
                                nc.tensor.matmul(
                                    out=avps[:, hi, :],
                                    lhsT=r(vo[:, jb, h, :]),
                                    rhs=r(aw[:, jb, :]),
                                    start=(jb == 0), stop=(jb == 1))
                        if pr % 2 == 0:
                            nc.vector.tensor_copy(
                                out=usb_row[:, he:ho + 1, :], in_=avps)
                        else:
                            nc.scalar.copy(
                                out=usb_row[:, he:ho + 1, :], in_=avps)

                    # softmax denominators: single gather of row 64 -> [8, N]
                    sums = smal.tile([H, N], F32, tag="sums")
                    nc.sync.dma_start(out=sums, in_=usb_row[DH:DH + 1, :, :])
                    rm = smal.tile([H, N], F32, tag="rm")
                    nc.vector.reciprocal(out=rm, in_=sums)
                    rm_d = rdram.tile([H, N], F32, name="rm_d")
                    nc.sync.dma_start(out=rm_d, in_=rm)

                    yps = psum.tile([128, 2 * N], F32, tag="ypq", bufs=2,
                                    name="yps")
                    # rbc_all[p, pr, i]: rows 0:64 = 1/sums[2pr], 64:128 = 2pr+1
                    rbc_all = att2.tile([128, NPAIR, N], F32, tag="rbc", bufs=2,
                                        name="rbc_all")
                    rmap = rm_d[:]
                    for sub in range(2):
                        nc.sync.dma_start(
                            out=rbc_all[sub * 64:(sub + 1) * 64, :, :],
                            in_=bass.AP(tensor=rmap.tensor,
                                        offset=rmap.offset + sub * N,
                                        ap=[[0, 64], [2 * N, NPAIR], [1, N]]))
                    ts = []
                    for pr in range(NPAIR):
                        he, ho = 2 * pr, 2 * pr + 1
                        up = att2.tile([128, N], F32, tag="up", name="up")
                        nc.sync.dma_start(out=up[0:64, :],
                                          in_=usb_row[0:DH, he, :])
                        nc.sync.dma_start(out=up[64:128, :],
                                          in_=usb_row[0:DH, ho, :])
                        w = att2.tile([128, N], F32, tag="w", name="w")
                        nc.gpsimd.tensor_mul(out=w, in0=up,
                                             in1=rbc_all[:, pr, :])
                        t = att2.tile([128, N], F32R, tag="t", name="t")
                        nc.vector.scalar_tensor_tensor(
                            out=t, in0=w, scalar=bv_sb[:, pr:pr + 1],
                            in1=gT[:, pr, :], op0=ALU.add, op1=ALU.mult)
                        ts.append(t)
                    for dc in range(2):
                        for pr in range(NPAIR):
                            nc.tensor.matmul(
                                out=yps[:, dc * N:(dc + 1) * N],
                                lhsT=r(wo_sb[:, pr, dc * 128:(dc + 1) * 128]),
                                rhs=r(ts[pr]),
                                start=(pr == 0), stop=(pr == NPAIR - 1))
                    for dc in range(2):
                        ysb = smal.tile([128, N], F32, tag="ysb")
                        nc.vector.tensor_scalar_add(
                            out=ysb, in0=yps[:, dc * N:(dc + 1) * N],
                            scalar1=bo_sb[:, dc:dc + 1])
                        nc.gpsimd.dma_start(
                            out=y_out[m, dc * 128:(dc + 1) * 128, :], in_=ysb)

                # prime the projection pipeline before qm blocks the PE stream
                tiles = {}
                for m in range(min(proj_pipe, m_loc)):
                    tiles[m] = proj(m)

                # bias^T readback + exp
                nc.sync.dma_start(
                    out=ebt_bf.rearrange("p a h (c i) -> p a h c i", c=n_cores),
                    in_=src)
                nc.scalar.activation(out=ebt, in_=ebt_bf, func=AF.Exp)

                # tied queries: qm^T = Wq'^T @ xnm^T  (+ bq)
                for eb in range(4):
                    qps = psum.tile([128, N], F32, tag="ypq", bufs=2, name="qps",
                                    padded_shape=[128, 2 * N])
                    for db in range(2):
                        nc.tensor.matmul(
                            out=qps,
                            lhsT=r(wq_sb[:, db, eb * 128:(eb + 1) * 128]),
                            rhs=r(xnmT[:, db, :]),
                            start=(db == 0), stop=(db == 1))
                    nc.scalar.activation(out=qmT[:, eb, :], in_=qps,
                                         func=AF.Identity, bias=bq_sb[:, eb:eb + 1])

                for m in range(m_loc):
                    attn(m, *tiles.pop(m))
                    nm = m + proj_pipe
                    if nm < m_loc:
                        tiles[nm] = proj(nm)

    if split_waits:
        _split_multi_waits(nc)
    return nc


def prep_inputs(x, edges, ln_g, ln_b, Wq, Wkv, Wg, bg, Wo, bo, Wb,
                n_cores: int = NCORES):
    """Host-side prep: fold LayerNorm affine into the projections, shard."""
    scale = DH ** -0.5
    g = ln_g.astype(np.float32)
    b = ln_b.astype(np.float32)
    wk = (g[:, None] * Wkv[:, :INNER]).astype(np.float32)
    wv = (g[:, None] * Wkv[:, INNER:]).astype(np.float32)
    wg = (g[:, None] * Wg).astype(np.float32)
    wq = (g[:, None] * Wq * (scale / M)).astype(np.float32)
    bk = (b @ Wkv[:, :INNER]).astype(np.float32)
    bv = (b @ Wkv[:, INNER:]).astype(np.float32)
    bgf = ((bg + b @ Wg) / 2).astype(np.float32)  # halved: tanh gate path
    bq = ((b @ Wq) * scale).astype(np.float32)

    m_loc = M // n_cores
    i_loc = N // n_cores
    sel = np.zeros((NPAIR, H, 128), np.float32)
    for pr in range(NPAIR):
        sel[pr, 2 * pr, 0:64] = 1.0
        sel[pr, 2 * pr + 1, 64:128] = 1.0
    shared = dict(sel=sel, wk=wk, wv=wv, wg=wg, wq=wq,
                  wo=np.ascontiguousarray(Wo, np.float32),
                  wb=np.ascontiguousarray(Wb, np.float32),
                  bk=bk, bv=bv, bg=bgf, bq=bq,
                  bo=np.ascontiguousarray(bo, np.float32))
    in_maps = []
    for c in range(n_cores):
        im = dict(shared)
        im["x"] = np.ascontiguousarray(x[0, c * m_loc:(c + 1) * m_loc], np.float32)
        im["edges"] = np.ascontiguousarray(
            edges[0, c * i_loc:(c + 1) * i_loc].reshape(i_loc * N, DE), np.float32)
        in_maps.append(im)
    return in_maps


def kernel(x, edges, mask, ln_g, ln_b, Wq, Wkv, Wg, bg, Wo, bo, Wb):
    """Full-input entry point: shard, run on 8 NeuronCores, gather."""
    del mask  # all-ones per the problem spec; softmax unmasked
    from concourse.bass_utils import run_bass_kernel_spmd

    x = np.asarray(x)
    nc = build_program(NCORES, M_LOC)
    in_maps = prep_inputs(np.asarray(x), np.asarray(edges), np.asarray(ln_g),
                          np.asarray(ln_b), np.asarray(Wq), np.asarray(Wkv),
                          np.asarray(Wg), np.asarray(bg), np.asarray(Wo),
                          np.asarray(bo), np.asarray(Wb))
    res = run_bass_kernel_spmd(nc, in_maps, list(range(NCORES)))
    outs = [res.results[c]["y"] for c in range(NCORES)]
    y = np.concatenate(outs, axis=0)          # [M, D, N]
    y = np.ascontiguousarray(np.transpose(y, (0, 2, 1)))  # [M, N, D]
    return y.reshape(B, M, N, D).astype(np.float32)

